# revision 18
# baseline (speedup 1.0000x reference)
"""ARMA2d Trainium2 kernel: conv3x3 (256->256) + per-channel circular AR
solve, data-parallel over batch across 8 NeuronCores.

Math: reference does y = conv3x3(x, w); then per channel c a circular
2D AR solve y <- ifft2(fft2(y)/fft2(a_c)) where a_c is a separable 3x3
circular filter derived from alpha. Since a_c is separable, the solve is
out[b,c] = Gh[c] @ y[b,c] @ Gw[c].T with Gh/Gw 64x64 circulant-inverse
matrices (exact, precomputed on host from alpha in float64).

Device work per core (4 images): conv as 18 shifted matmuls (2 ci-tiles
x 9 taps) accumulated in PSUM; AR as PE transposes + two matmuls per
channel batched over the 4 images (N=256).
"""
import sys
import numpy as np

if "/opt/trn_rl_repo" not in sys.path:
    sys.path.insert(0, "/opt/trn_rl_repo")

B, C, H, W = 32, 256, 64, 64
NCORES = 8
BP = B // NCORES  # images per core

_CACHE = {}
LAST_EXEC_NS = None


def _build_nc(use_f32r=True):
    from contextlib import ExitStack
    import concourse.tile as tile
    from concourse import mybir, bacc

    f32 = mybir.dt.float32
    DT = mybir.dt.float32r if use_f32r else f32

    nc = bacc.Bacc("TRN2", target_bir_lowering=False, debug=False,
                   num_devices=NCORES)
    xp_p = nc.declare_dram_parameter("xp", [BP, 2, 128, 66, 66], DT, isOutput=False)
    wt_p = nc.declare_dram_parameter("wt", [128, 3, 3, 2, 2, 128], DT, isOutput=False)
    gwt_p = nc.declare_dram_parameter("gwt", [128, C // 2, 128], DT, isOutput=False)
    ght_p = nc.declare_dram_parameter("ght", [64, C, 64], DT, isOutput=False)
    id_p = nc.declare_dram_parameter("ident", [128, 128], DT, isOutput=False)
    out_p = nc.declare_dram_parameter("out", [BP, C, H, W], DT, isOutput=True)

    with tile.TileContext(nc) as tc, ExitStack() as ctx:
        consts = ctx.enter_context(tc.tile_pool(name="consts", bufs=1))
        w_sb = consts.tile([128, 3, 3, 2, 2, 128], DT)
        nc.sync.dma_start(w_sb[:], wt_p[:])
        id_sb = consts.tile([128, 128], DT)
        nc.sync.dma_start(id_sb[:], id_p[:])

        dram = ctx.enter_context(tc.tile_pool(name="dram", bufs=1, space="DRAM"))
        y_hbm = dram.tile([2, 128, BP, H, W], DT)  # [cot, cl, b, h, w]

        xpool = ctx.enter_context(tc.tile_pool(name="xpool", bufs=2))

        # ---- Phase 1+2 interleaved: conv per cot, then AR chunks whose
        # channels that cot completes (keeps PE dense: AR stalls are filled
        # by conv matmuls of the other half).
        ypool = ctx.enter_context(tc.tile_pool(name="ypool", bufs=3))
        cpsum = ctx.enter_context(tc.tile_pool(name="cpsum", bufs=2, space="PSUM"))

        CH = 32   # channels per G chunk
        QPC = CH // 4
        gpool = ctx.enter_context(tc.tile_pool(name="gpool", bufs=2))
        arp = ctx.enter_context(tc.tile_pool(name="arp", bufs=3))
        psA = ctx.enter_context(tc.tile_pool(name="psA", bufs=1, space="PSUM"))
        psB = ctx.enter_context(tc.tile_pool(name="psB", bufs=1, space="PSUM"))
        psC = ctx.enter_context(tc.tile_pool(name="psC", bufs=2, space="PSUM"))
        psD = ctx.enter_context(tc.tile_pool(name="psD", bufs=2, space="PSUM"))
        id64 = id_sb[0:64, 0:64]

        def conv_cot(cot):
            for b in range(BP):
                xts = []
                for cit in range(2):
                    xt = xpool.tile([128, 66, 66], DT, tag=f"x{cit}",
                                    name=f"x{cit}_{cot}_{b}")
                    nc.sync.dma_start(xt[:], xp_p[b, cit])
                    xts.append(xt)
                for rb in range(8):
                    ps = cpsum.tile([128, 8, 64], f32, tag="cps",
                                    name=f"ps_{b}_{cot}_{rb}")
                    k = 0
                    for cit in range(2):
                        for ky in range(3):
                            for kx in range(3):
                                lhsT = w_sb[:, ky, kx, cit, cot, :]
                                rhs = xts[cit][:, rb * 8 + ky: rb * 8 + ky + 8,
                                               kx: kx + 64]
                                nc.tensor.matmul(ps[:], lhsT, rhs,
                                                 start=(k == 0), stop=(k == 17))
                                k += 1
                    ysb = ypool.tile([128, 8, 64], DT, tag="ysb",
                                     name=f"ysb_{b}_{cot}_{rb}")
                    nc.scalar.copy(ysb[:], ps[:])
                    nc.sync.dma_start(y_hbm[cot, :, b, rb * 8:(rb + 1) * 8, :], ysb[:])

        def ar_chunk(chunk):
            gwbd = gpool.tile([128, CH // 2, 128], DT, tag="gw", name=f"gw_{chunk}")
            nc.sync.dma_start(gwbd[:], gwt_p[:, chunk * (CH // 2):(chunk + 1) * (CH // 2), :])
            gh_sb = gpool.tile([64, CH, 64], DT, tag="gh", name=f"gh_{chunk}")
            nc.sync.dma_start(gh_sb[:], ght_p[:, chunk * CH:(chunk + 1) * CH, :])

            for qd in range(QPC):
                qbase = chunk * CH + qd * 4
                yq = arp.tile([64, 2, BP, 2, 64], DT, tag="yq", name=f"yq_{qbase}")
                for q in range(2):
                    for par in range(2):
                        c = qbase + 2 * q + par
                        nc.gpsimd.dma_start(
                            yq[:, q, :, par, :],
                            y_hbm[c >> 7, c & 127].transpose([1, 0, 2]))
                # T1 per (q,b): [64,(c,w)=128] -> [(c,w)=128, h=64]
                t1q = psA.tile([128, 2, BP, 64], f32, tag="t1", name=f"t1_{qbase}")
                for q in range(2):
                    for bb in range(BP):
                        nc.tensor.transpose(t1q[:, q, bb, :].bitcast(DT),
                                            yq[:, q, bb, :, :], id64)
                t1s = arp.tile([128, 2, BP, 64], DT, tag="t1s", name=f"t1s_{qbase}")
                nc.scalar.copy(t1s[:], t1q[:])
                # mm1 block-diag per pair
                p2q = psB.tile([128, 2, BP, 64], f32, tag="p2", name=f"p2_{qbase}")
                for q in range(2):
                    pl = (qbase - chunk * CH) // 2 + q
                    nc.tensor.matmul(p2q[:, q], gwbd[:, pl, :],
                                     t1s[:, q, :, :], start=True, stop=True)
                p2s = arp.tile([128, 2, BP, 64], DT, tag="p2s", name=f"p2s_{qbase}")
                nc.scalar.copy(p2s[:], p2q[:])
                # per pair: T2 + mm2 + out
                for q in range(2):
                    t2q = psC.tile([64, BP, 128], f32, tag="t2", name=f"t2_{qbase}_{q}")
                    for bb in range(BP):
                        nc.tensor.transpose(t2q[:, bb, :].bitcast(DT),
                                            p2s[:, q, bb, :], id_sb[:])
                    t2s = arp.tile([64, 2, BP, 64], DT, tag=f"t2s{q}",
                                   name=f"t2s_{qbase}_{q}")
                    nc.vector.tensor_copy(
                        t2s[:].transpose([0, 2, 1, 3]),
                        t2q[:].rearrange("h b (p j) -> h b p j", p=2))
                    vq = psD.tile([64, 2, BP, 64], f32, tag="v", name=f"v_{qbase}_{q}")
                    for par in range(2):
                        cl = (qbase - chunk * CH) + 2 * q + par
                        nc.tensor.matmul(vq[:, par], gh_sb[:, cl, :],
                                         t2s[:, par], start=True, stop=True)
                    vs = arp.tile([64, 2, BP, 64], DT, tag=f"vs{q}",
                                  name=f"vs_{qbase}_{q}")
                    nc.vector.tensor_copy(vs[:], vq[:])
                    for par in range(2):
                        c = qbase + 2 * q + par
                        nc.sync.dma_start(out_p[:, c].transpose([1, 0, 2]),
                                          vs[:, par])

        conv_cot(0)
        for chunk in range(4):
            ar_chunk(chunk)
        conv_cot(1)
        for chunk in range(4, 8):
            ar_chunk(chunk)

    nc.compile()
    return nc


def _host_prep(x, w, alpha):
    # circulant-inverse matrices from alpha (float64 for stability)
    s = np.sin(-np.pi / 4.0)
    c = np.cos(-np.pi / 4.0)
    aw = np.zeros((2, 3), dtype=np.float64)
    aw[0, 0] = np.float32(c)
    aw[1, 0] = np.float32(-s)
    aw[0, -1] = np.float32(s)
    aw[1, -1] = np.float32(c)
    at = np.tanh(alpha.astype(np.float64))  # (C,1,2,2)
    A_xy = np.einsum("ckab,bj->ckaj", at, aw)
    A_xy[:, :, :, 1] = 1.0
    A_x = A_xy[:, 0, 0, :]  # taps along H
    A_y = A_xy[:, 0, 1, :]  # taps along W

    def circ_inv(taps):
        a = np.zeros((taps.shape[0], H), dtype=np.float64)
        a[:, -1] = taps[:, 0]
        a[:, 0] = taps[:, 1]
        a[:, 1] = taps[:, 2]
        F = np.fft.fft(a, axis=-1)
        g = np.real(np.fft.ifft(1.0 / F, axis=-1))
        idx = (np.arange(H)[:, None] - np.arange(H)[None, :]) % H
        return g[:, idx]  # (C,64,64): out = G @ y

    Gh = circ_inv(A_x)
    Gw = circ_inv(A_y)
    # gwt: block-diag pairs [128, C//2, 128]:
    # rows 0-63 x cols 0-63 = GwT[2p] ([w,j]); rows/cols 64-127 = GwT[2p+1]
    gwT = Gw.transpose(0, 2, 1).astype(np.float32)   # [c][w, j]
    gwt = np.zeros((128, C // 2, 128), dtype=np.float32)
    for p in range(C // 2):
        gwt[0:64, p, 0:64] = gwT[2 * p]
        gwt[64:128, p, 64:128] = gwT[2 * p + 1]
    # ght compact: [h, c, i]
    ght = np.ascontiguousarray(Gh.transpose(2, 0, 1)).astype(np.float32)

    # weights: [ci_l, ky, kx, cit, cot, co_l]
    wt = w.reshape(2, 128, 2, 128, 3, 3).transpose(3, 4, 5, 2, 0, 1)
    wt = np.ascontiguousarray(wt).astype(np.float32)

    # x shards, padded
    xr = x.reshape(NCORES, BP, 2, 128, H, W)
    xpad = np.zeros((NCORES, BP, 2, 128, 66, 66), dtype=np.float32)
    xpad[..., 1:65, 1:65] = xr

    ident = np.eye(128, dtype=np.float32)
    return xpad, wt, gwt, ght, ident


def kernel(x, w, alpha, _trace=False):
    global LAST_EXEC_NS
    from concourse.bass_utils import run_bass_kernel_spmd

    x = np.ascontiguousarray(np.asarray(x), dtype=np.float32)
    w = np.ascontiguousarray(np.asarray(w), dtype=np.float32)
    alpha = np.asarray(alpha).astype(np.float64)

    key = "nc"
    if key not in _CACHE:
        _CACHE[key] = _build_nc()
    nc = _CACHE[key]

    xpad, wt, gwt, ght, ident = _host_prep(x, w, alpha)
    in_maps = [
        {"xp": xpad[i], "wt": wt, "gwt": gwt, "ght": ght, "ident": ident}
        for i in range(NCORES)
    ]
    res = run_bass_kernel_spmd(nc, in_maps, core_ids=list(range(NCORES)),
                               trace=_trace)
    LAST_EXEC_NS = res.exec_time_ns
    _CACHE["last_res"] = res
    outs = [np.asarray(r["out"]) for r in res.results]
    return np.concatenate(outs, axis=0)


# revision 19
# speedup vs baseline: 1.0084x; 1.0084x over previous
"""ARMA2d Trainium2 kernel: conv3x3 (256->256) + per-channel circular AR
solve, data-parallel over batch across 8 NeuronCores.

Math: reference does y = conv3x3(x, w); then per channel c a circular
2D AR solve y <- ifft2(fft2(y)/fft2(a_c)) where a_c is a separable 3x3
circular filter derived from alpha. Since a_c is separable, the solve is
out[b,c] = Gh[c] @ y[b,c] @ Gw[c].T with Gh/Gw 64x64 circulant-inverse
matrices (exact, precomputed on host from alpha in float64).

Device work per core (4 images): conv as 18 shifted matmuls (2 ci-tiles
x 9 taps) accumulated in PSUM; AR as PE transposes + two matmuls per
channel batched over the 4 images (N=256).
"""
import sys
import numpy as np

if "/opt/trn_rl_repo" not in sys.path:
    sys.path.insert(0, "/opt/trn_rl_repo")

B, C, H, W = 32, 256, 64, 64
NCORES = 8
BP = B // NCORES  # images per core

_CACHE = {}
LAST_EXEC_NS = None


def _build_nc(use_f32r=True):
    from contextlib import ExitStack
    import concourse.tile as tile
    from concourse import mybir, bacc

    f32 = mybir.dt.float32
    DT = mybir.dt.float32r if use_f32r else f32

    nc = bacc.Bacc("TRN2", target_bir_lowering=False, debug=False,
                   num_devices=NCORES)
    xp_p = nc.declare_dram_parameter("xp", [BP, 2, 128, 66, 66], DT, isOutput=False)
    wt_p = nc.declare_dram_parameter("wt", [128, 3, 3, 2, 2, 128], DT, isOutput=False)
    gwt_p = nc.declare_dram_parameter("gwt", [128, C // 2, 128], DT, isOutput=False)
    ght_p = nc.declare_dram_parameter("ght", [64, C, 64], DT, isOutput=False)
    id_p = nc.declare_dram_parameter("ident", [128, 128], DT, isOutput=False)
    out_p = nc.declare_dram_parameter("out", [BP, C, H, W], DT, isOutput=True)

    with tile.TileContext(nc) as tc, ExitStack() as ctx:
        consts = ctx.enter_context(tc.tile_pool(name="consts", bufs=1))
        w_sb = consts.tile([128, 3, 3, 2, 2, 128], DT)
        nc.sync.dma_start(w_sb[:], wt_p[:])
        id_sb = consts.tile([128, 128], DT)
        nc.sync.dma_start(id_sb[:], id_p[:])

        dram = ctx.enter_context(tc.tile_pool(name="dram", bufs=1, space="DRAM"))
        y_hbm = dram.tile([2, 128, BP, H, W], DT)  # [cot, cl, b, h, w]

        xpool = ctx.enter_context(tc.tile_pool(name="xpool", bufs=2))

        # ---- Phase 1+2 interleaved: conv per cot, then AR chunks whose
        # channels that cot completes (keeps PE dense: AR stalls are filled
        # by conv matmuls of the other half).
        ypool = ctx.enter_context(tc.tile_pool(name="ypool", bufs=3))
        cpsum = ctx.enter_context(tc.tile_pool(name="cpsum", bufs=2, space="PSUM"))

        CH = 32   # channels per G chunk
        QPC = CH // 4
        gpool = ctx.enter_context(tc.tile_pool(name="gpool", bufs=2))
        arp = ctx.enter_context(tc.tile_pool(name="arp", bufs=3))
        psA = ctx.enter_context(tc.tile_pool(name="psA", bufs=1, space="PSUM"))
        psB = ctx.enter_context(tc.tile_pool(name="psB", bufs=1, space="PSUM"))
        psC = ctx.enter_context(tc.tile_pool(name="psC", bufs=2, space="PSUM"))
        psD = ctx.enter_context(tc.tile_pool(name="psD", bufs=2, space="PSUM"))
        id64 = id_sb[0:64, 0:64]

        def conv_cot(cot):
            for b in range(BP):
                xts = []
                for cit in range(2):
                    xt = xpool.tile([128, 66, 66], DT, tag=f"x{cit}",
                                    name=f"x{cit}_{cot}_{b}")
                    nc.sync.dma_start(xt[:], xp_p[b, cit])
                    xts.append(xt)
                for rb in range(8):
                    ps = cpsum.tile([128, 8, 64], f32, tag="cps",
                                    name=f"ps_{b}_{cot}_{rb}")
                    k = 0
                    for cit in range(2):
                        for ky in range(3):
                            for kx in range(3):
                                lhsT = w_sb[:, ky, kx, cit, cot, :]
                                rhs = xts[cit][:, rb * 8 + ky: rb * 8 + ky + 8,
                                               kx: kx + 64]
                                nc.tensor.matmul(ps[:], lhsT, rhs,
                                                 start=(k == 0), stop=(k == 17))
                                k += 1
                    ysb = ypool.tile([128, 8, 64], DT, tag="ysb",
                                     name=f"ysb_{b}_{cot}_{rb}")
                    nc.scalar.copy(ysb[:], ps[:])
                    nc.sync.dma_start(y_hbm[cot, :, b, rb * 8:(rb + 1) * 8, :], ysb[:])

        def ar_chunk(chunk):
            gwbd = gpool.tile([128, CH // 2, 128], DT, tag="gw", name=f"gw_{chunk}")
            nc.sync.dma_start(gwbd[:], gwt_p[:, chunk * (CH // 2):(chunk + 1) * (CH // 2), :])
            gh_sb = gpool.tile([64, CH, 64], DT, tag="gh", name=f"gh_{chunk}")
            nc.sync.dma_start(gh_sb[:], ght_p[:, chunk * CH:(chunk + 1) * CH, :])

            for qd in range(QPC):
                qbase = chunk * CH + qd * 4
                yq = arp.tile([64, 2, BP, 2, 64], DT, tag="yq", name=f"yq_{qbase}")
                for q in range(2):
                    for par in range(2):
                        c = qbase + 2 * q + par
                        nc.gpsimd.dma_start(
                            yq[:, q, :, par, :],
                            y_hbm[c >> 7, c & 127].transpose([1, 0, 2]))
                # T1 per (q,b): [64,(c,w)=128] -> [(c,w)=128, h=64]
                t1q = psA.tile([128, 2, BP, 64], f32, tag="t1", name=f"t1_{qbase}")
                for q in range(2):
                    for bb in range(BP):
                        nc.tensor.transpose(t1q[:, q, bb, :].bitcast(DT),
                                            yq[:, q, bb, :, :], id64)
                t1s = arp.tile([128, 2, BP, 64], DT, tag="t1s", name=f"t1s_{qbase}")
                nc.vector.tensor_copy(t1s[:], t1q[:])
                # mm1 block-diag per pair
                p2q = psB.tile([128, 2, BP, 64], f32, tag="p2", name=f"p2_{qbase}")
                for q in range(2):
                    pl = (qbase - chunk * CH) // 2 + q
                    nc.tensor.matmul(p2q[:, q], gwbd[:, pl, :],
                                     t1s[:, q, :, :], start=True, stop=True)
                p2s = arp.tile([128, 2, BP, 64], DT, tag="p2s", name=f"p2s_{qbase}")
                nc.vector.tensor_copy(p2s[:], p2q[:])
                # per pair: T2 + mm2 + out
                for q in range(2):
                    t2q = psC.tile([64, BP, 128], f32, tag="t2", name=f"t2_{qbase}_{q}")
                    for bb in range(BP):
                        nc.tensor.transpose(t2q[:, bb, :].bitcast(DT),
                                            p2s[:, q, bb, :], id_sb[:])
                    t2s = arp.tile([64, 2, BP, 64], DT, tag=f"t2s{q}",
                                   name=f"t2s_{qbase}_{q}")
                    nc.vector.tensor_copy(
                        t2s[:].transpose([0, 2, 1, 3]),
                        t2q[:].rearrange("h b (p j) -> h b p j", p=2))
                    vq = psD.tile([64, 2, BP, 64], f32, tag="v", name=f"v_{qbase}_{q}")
                    for par in range(2):
                        cl = (qbase - chunk * CH) + 2 * q + par
                        nc.tensor.matmul(vq[:, par], gh_sb[:, cl, :],
                                         t2s[:, par], start=True, stop=True)
                    vs = arp.tile([64, 2, BP, 64], DT, tag=f"vs{q}",
                                  name=f"vs_{qbase}_{q}")
                    nc.vector.tensor_copy(vs[:], vq[:])
                    for par in range(2):
                        c = qbase + 2 * q + par
                        nc.sync.dma_start(out_p[:, c].transpose([1, 0, 2]),
                                          vs[:, par])

        conv_cot(0)
        for chunk in range(4):
            ar_chunk(chunk)
        conv_cot(1)
        for chunk in range(4, 8):
            ar_chunk(chunk)

    nc.compile()
    return nc


def _host_prep(x, w, alpha):
    # circulant-inverse matrices from alpha (float64 for stability)
    s = np.sin(-np.pi / 4.0)
    c = np.cos(-np.pi / 4.0)
    aw = np.zeros((2, 3), dtype=np.float64)
    aw[0, 0] = np.float32(c)
    aw[1, 0] = np.float32(-s)
    aw[0, -1] = np.float32(s)
    aw[1, -1] = np.float32(c)
    at = np.tanh(alpha.astype(np.float64))  # (C,1,2,2)
    A_xy = np.einsum("ckab,bj->ckaj", at, aw)
    A_xy[:, :, :, 1] = 1.0
    A_x = A_xy[:, 0, 0, :]  # taps along H
    A_y = A_xy[:, 0, 1, :]  # taps along W

    def circ_inv(taps):
        a = np.zeros((taps.shape[0], H), dtype=np.float64)
        a[:, -1] = taps[:, 0]
        a[:, 0] = taps[:, 1]
        a[:, 1] = taps[:, 2]
        F = np.fft.fft(a, axis=-1)
        g = np.real(np.fft.ifft(1.0 / F, axis=-1))
        idx = (np.arange(H)[:, None] - np.arange(H)[None, :]) % H
        return g[:, idx]  # (C,64,64): out = G @ y

    Gh = circ_inv(A_x)
    Gw = circ_inv(A_y)
    # gwt: block-diag pairs [128, C//2, 128]:
    # rows 0-63 x cols 0-63 = GwT[2p] ([w,j]); rows/cols 64-127 = GwT[2p+1]
    gwT = Gw.transpose(0, 2, 1).astype(np.float32)   # [c][w, j]
    gwt = np.zeros((128, C // 2, 128), dtype=np.float32)
    for p in range(C // 2):
        gwt[0:64, p, 0:64] = gwT[2 * p]
        gwt[64:128, p, 64:128] = gwT[2 * p + 1]
    # ght compact: [h, c, i]
    ght = np.ascontiguousarray(Gh.transpose(2, 0, 1)).astype(np.float32)

    # weights: [ci_l, ky, kx, cit, cot, co_l]
    wt = w.reshape(2, 128, 2, 128, 3, 3).transpose(3, 4, 5, 2, 0, 1)
    wt = np.ascontiguousarray(wt).astype(np.float32)

    # x shards, padded
    xr = x.reshape(NCORES, BP, 2, 128, H, W)
    xpad = np.zeros((NCORES, BP, 2, 128, 66, 66), dtype=np.float32)
    xpad[..., 1:65, 1:65] = xr

    ident = np.eye(128, dtype=np.float32)
    return xpad, wt, gwt, ght, ident


def kernel(x, w, alpha, _trace=False):
    global LAST_EXEC_NS
    from concourse.bass_utils import run_bass_kernel_spmd

    x = np.ascontiguousarray(np.asarray(x), dtype=np.float32)
    w = np.ascontiguousarray(np.asarray(w), dtype=np.float32)
    alpha = np.asarray(alpha).astype(np.float64)

    key = "nc"
    if key not in _CACHE:
        _CACHE[key] = _build_nc()
    nc = _CACHE[key]

    xpad, wt, gwt, ght, ident = _host_prep(x, w, alpha)
    in_maps = [
        {"xp": xpad[i], "wt": wt, "gwt": gwt, "ght": ght, "ident": ident}
        for i in range(NCORES)
    ]
    res = run_bass_kernel_spmd(nc, in_maps, core_ids=list(range(NCORES)),
                               trace=_trace)
    LAST_EXEC_NS = res.exec_time_ns
    _CACHE["last_res"] = res
    outs = [np.asarray(r["out"]) for r in res.results]
    return np.concatenate(outs, axis=0)
